# revision 49
# baseline (speedup 1.0000x reference)
"""DeepNCMDecoder Trainium2 kernel: 8-core data-parallel over batch.

Per core (batch shard BL=1024):
  h   = relu(x @ W1 + b1)           -- computed transposed: hT[n2, b]
  enc = h @ W2 + b2                 -- computed transposed: encT[e, b]
  scores = -max(x2 + p2 - 2 enc@P^T, 0) = min(2*s - x2 - p2, 0)
  updates = one_hot(labels).T @ enc   (partial; host sums across cores)
  counts  = bincount(labels)          (host; pure label metadata)

All matmuls run as fp32r (TF32-class, ~2e-4 rel err, full PE rate).
"""
import numpy as np
import concourse.bass as bass
import concourse.mybir as mybir
import concourse.tile as tile
from concourse import bacc
from concourse.bass_utils import run_bass_kernel_spmd
from concourse.masks import make_identity

F32 = mybir.dt.float32
F32R = mybir.dt.float32r
AF = mybir.ActivationFunctionType
ALU = mybir.AluOpType

B, D, E, C = 8192, 1024, 1024, 1000
N2 = 2 * E               # 2048
NCORES = 8
BL = B // NCORES         # 1024 rows per core
P = 128

_CACHED = {}             # variant -> compiled Bacc program
LAST_RESULTS = None      # test harness introspection
WINDOWS = (256, 384)     # updates window ladder (2 or 3 c-tiles); host sorts
CWIN = 384               # widest windowed variant (for the host pad buffer)


def build_nc(phases="1234", cwin=256):
    nc = bacc.Bacc("TRN2", target_bir_lowering=False, debug=False,
                   num_devices=NCORES)

    xT = nc.dram_tensor("xT", [D, BL], F32, kind="ExternalInput")
    w1p = nc.dram_tensor("w1p", [P, N2 // P, D // P, P], F32, kind="ExternalInput")
    w2p = nc.dram_tensor("w2p", [P, E // P, N2 // P, P], F32, kind="ExternalInput")
    pT = nc.dram_tensor("pT", [E, C], F32, kind="ExternalInput")
    p2r = nc.dram_tensor("p2r", [P, C], F32, kind="ExternalInput")
    b1c = nc.dram_tensor("b1c", [P, N2 // P], F32, kind="ExternalInput")
    b2c = nc.dram_tensor("b2c", [P, E // P], F32, kind="ExternalInput")
    labf = nc.dram_tensor("labf", [P, BL // P], F32, kind="ExternalInput")

    CU = cwin if cwin else C          # updates window width
    scores = nc.dram_tensor("scores", [BL, C], F32, kind="ExternalOutput")
    upd = nc.dram_tensor("upd", [CU, E], F32, kind="ExternalOutput")

    KD = D // P          # 8  k-tiles for mm1
    MT = N2 // P         # 16 m-tiles (hT partition tiles)
    KN2 = N2 // P        # 16 k-tiles for mm2
    ET = E // P          # 8  e-tiles (encT partition tiles)
    BT = BL // P         # 8  b-tiles
    NB = BL // 512       # 2  n-tiles over b
    CH = C // 2          # 500 (c split for PSUM)
    CT_SIZES = [P] * (CU // P) + ([CU % P] if CU % P else [])

    with tile.TileContext(nc) as tc:
        with (
            tc.tile_pool(name="persist", bufs=1) as persist,
            tc.tile_pool(name="psum", bufs=2, space="PSUM") as psum,
        ):
            # --- persistent small tensors -------------------------------
            b1_sb = persist.tile([P, MT], F32, name="b1_sb")
            pT_sb = persist.tile([P, ET, C], F32R, name="pT_sb")
            b2_sb = persist.tile([P, ET], F32, name="b2_sb")
            encT = persist.tile([P, ET, E], F32R, name="encT")
            ident = persist.tile([P, P], F32, name="ident")
            x2pos = persist.tile([P, BT], F32, name="x2pos")

            do2 = "2" in phases
            do3 = "3" in phases
            do4 = "4" in phases

            ph_hT = tc.alloc_tile_pool(name="ph_hT", bufs=1)
            wpool = tc.alloc_tile_pool(name="wpool", bufs=4)
            hT = ph_hT.tile([P, MT, BL], F32R, name="hT")

            def load_w1(m):
                wt = wpool.tile([P, KN2, P], F32R, name="wt", tag="wt")
                nc.sync.dma_start(wt[:, :KD, :], w1p[:, m].bitcast(F32R))
                return wt

            def load_w2(e):
                wt = wpool.tile([P, KN2, P], F32R, name="wt", tag="wt")
                nc.sync.dma_start(wt, w2p[:, e].bitcast(F32R))
                return wt

            # --- mm1: hT = relu(W1.T x.T + b1) ----------------------
            ph_mm1 = tc.alloc_tile_pool(name="ph_mm1", bufs=1)
            # PE warmup: dummy matmuls during the startup DMA window keep the
            # PE continuously busy (model ramp + HW HAM) until real data lands
            wu = ph_mm1.tile([P, 512], F32, name="wu")
            nc.vector.memset(wu, 1.0)
            for i in range(8):
                wps = psum.tile([P, 512], F32, name=f"wps_{i}", tag="tps")
                nc.tensor.matmul(wps, wu[:, 0:P].bitcast(F32R),
                                 wu.bitcast(F32R), start=True, stop=True)
            xT_sb = ph_mm1.tile([P, KD, BL], F32R, name="xT_sb")
            xT_r = xT.rearrange("(o p) b -> p o b", p=P).bitcast(F32R)
            # startup-critical loads first, split for latency
            nc.sync.dma_start(xT_sb[:, 0, 0:512], xT_r[:, 0, 0:512])
            w0 = wpool.tile([P, KN2, P], F32R, name="wt", tag="wt")
            nc.sync.dma_start(w0[:, 0:4, :], w1p[:, 0, 0:4].bitcast(F32R))
            nc.sync.dma_start(w0[:, 4:KD, :], w1p[:, 0, 4:KD].bitcast(F32R))
            w_next = w0
            # n-major xT streaming: first halves of every k, then second halves
            for k in range(1, KD):
                nc.sync.dma_start(xT_sb[:, k, 0:512], xT_r[:, k, 0:512])
            wq = [w_next, load_w1(1), load_w1(2)]
            nc.sync.dma_start(b1_sb, b1c[:, :])
            nc.sync.dma_start(b2_sb, b2c[:, :])
            for k in range(KD):
                nc.sync.dma_start(xT_sb[:, k, 512:BL], xT_r[:, k, 512:BL])
            make_identity(nc, ident)
            for m in range(MT):
                w1t = wq.pop(0)
                if m + 3 < MT:
                    wq.append(load_w1(m + 3))
                pss = [psum.tile([P, 512], F32, name=f"psA_{m}_{n}",
                                 tag=f"ps{n}", bufs=3) for n in range(NB)]
                for n in range(NB):
                    for k in range(KD):
                        nc.tensor.matmul(
                            pss[n], w1t[:, k, :],
                            xT_sb[:, k, n * 512:(n + 1) * 512],
                            start=(k == 0), stop=(k == KD - 1),
                        )
                for n in range(NB):
                    nc.scalar.activation(
                        hT[:, m, n * 512:(n + 1) * 512], pss[n],
                        AF.Relu, bias=b1_sb[:, m:m + 1], scale=1.0,
                    )
            ph_mm1.release()

            # enc lives on the right side of the heap: outlives hT/wpool
            ph_enc = tc.alloc_tile_pool(name="ph_enc", bufs=1, side="right")
            enc = ph_enc.tile([P, BT, E], F32R, name="enc")

            def transpose_etile(e):
                # PE-transpose encT e-tile into enc columns (one per b-tile)
                for bt in range(BT):
                    tps = psum.tile([P, P], F32, name=f"tps_{e}_{bt}",
                                    tag="tps")
                    nc.tensor.transpose(
                        tps, encT[:, e, bt * P:(bt + 1) * P].bitcast(F32),
                        ident,
                    )
                    nc.vector.tensor_copy(
                        enc[:, bt, e * P:(e + 1) * P], tps)

            # --- mm2: encT = W2.T hT + b2, transposes interleaved ----
            if do2:
                pT_r = pT.rearrange("(o p) c -> p o c", p=P).bitcast(F32R)
                w2q = [load_w2(0), load_w2(1)]
                for e in range(ET):
                    w2t = w2q.pop(0)
                    if e + 2 < ET:
                        w2q.append(load_w2(e + 2))
                    nc.sync.dma_start(pT_sb[:, e], pT_r[:, e])
                    pss = [psum.tile([P, 512], F32, name=f"psB_{e}_{n}",
                                     tag=f"ps{n}", bufs=3) for n in range(NB)]
                    for k in range(KN2):
                        for n in range(NB):
                            nc.tensor.matmul(
                                pss[n], w2t[:, k, :],
                                hT[:, k, n * 512:(n + 1) * 512],
                                start=(k == 0), stop=(k == KN2 - 1),
                            )
                    for n in range(NB):
                        nc.scalar.activation(
                            encT[:, e, n * 512:(n + 1) * 512], pss[n],
                            AF.Identity, bias=b2_sb[:, e:e + 1], scale=1.0,
                        )
                    if do4 and e > 0:
                        transpose_etile(e - 1)
                if do4:
                    transpose_etile(ET - 1)
            wpool.release()
            ph_hT.release()

            # --- phase 2: x2, one-hot, scores, updates ------------------
            with tc.tile_pool(name="ph2", bufs=1) as ph2, \
                 tc.tile_pool(name="scratch", bufs=2) as scratch:
                oh = ph2.tile([P, BT, CU], F32R, name="oh")
                p2_sb = ph2.tile([P, C], F32, name="p2_sb")
                lab_sb = ph2.tile([P, BT], F32, name="lab_sb")
                iota_sb = ph2.tile([P, C], F32, name="iota_sb")

                nc.sync.dma_start(p2_sb, p2r[:, :])
                nc.sync.dma_start(lab_sb, labf[:, :])
                nc.gpsimd.iota(iota_sb, [[1, C]], channel_multiplier=0,
                               allow_small_or_imprecise_dtypes=True)

                def emit_x2(bt):
                    # x2 on ACT (Square + accum_out); emitted behind each
                    # bt's psum-draining copies so the ACT FIFO stays prompt
                    scr = scratch.tile([P, E], F32, name="scr", tag="scr")
                    nc.scalar.activation(
                        scr, enc[:, bt, :].bitcast(F32), AF.Square,
                        accum_out=x2pos[:, bt:bt + 1],
                    )

                if do4:
                    emit_x2(0)
                else:
                    nc.vector.memset(x2pos, 0.0)

                # mm3: scores
                def emit_oh(bt):
                    nc.vector.tensor_scalar(
                        oh[:, bt, :], iota_sb[:, :CU], lab_sb[:, bt:bt + 1],
                        None, ALU.is_equal,
                    )

                for bt in range(BT if do3 else 0):
                    if do4 and bt < 4:
                        emit_oh(2 * bt)
                        emit_oh(2 * bt + 1)
                    pss = [psum.tile([P, 512], F32, name=f"psC_{bt}_{ci}",
                                     tag=f"ps{ci}", bufs=3) for ci in range(2)]
                    for k in range(ET):
                        lhs = encT[:, k, bt * P:(bt + 1) * P]
                        for ci in range(2):
                            nc.tensor.matmul(
                                pss[ci][:, :CH], lhs,
                                pT_sb[:, k, ci * CH:(ci + 1) * CH],
                                start=(k == 0), stop=(k == ET - 1),
                            )
                    sco = scratch.tile([P, C], F32, name="sco", tag="sco",
                                       bufs=3)
                    for ci in range(2):
                        nc.scalar.activation(
                            sco[:, ci * CH:(ci + 1) * CH], pss[ci][:, :CH],
                            AF.Copy, scale=2.0,
                        )
                    if do4 and bt + 1 < BT:
                        emit_x2(bt + 1)
                    nc.vector.tensor_tensor(sco, sco, p2_sb, ALU.subtract)
                    nc.vector.tensor_scalar(sco, sco, x2pos[:, bt:bt + 1],
                                            0.0, ALU.subtract, ALU.min)
                    nc.sync.dma_start(scores[bt * P:(bt + 1) * P, :], sco)

                if do4 and not do3:
                    for bt in range(BT):
                        emit_oh(bt)

                # mm4: updates + counts
                for ct, mct in enumerate(CT_SIZES if do4 else []):
                    # ct=0 uses the tps banks (idle after transposes) so the
                    # first mm4 group doesn't wait on mm3's psum slots
                    pss = [psum.tile([P, 512], F32, name=f"psD_{ct}_{n}",
                                     tag=("tps" if ct == 0 else f"ps{n}"),
                                     bufs=(2 if ct == 0 else 3))
                           for n in range(2)]
                    for bt in range(BT):
                        lhs = oh[:, bt, ct * P:ct * P + mct]
                        for n in range(2):
                            nc.tensor.matmul(
                                pss[n][:mct], lhs,
                                enc[:, bt, n * 512:(n + 1) * 512],
                                start=(bt == 0), stop=(bt == BT - 1),
                            )
                    usb = scratch.tile([P, E], F32, name="usb", tag="usb")
                    last = ct == len(CT_SIZES) - 1
                    for n in range(2):
                        if last and n == 1:
                            # final tile: DVE (idle at kernel end) drains the
                            # second half in parallel with ACT's first half,
                            # and its store rides the ACT HWDGE ring so the
                            # two final DMA configs parallelize
                            nc.vector.tensor_copy(
                                usb[:mct, 512:1024], pss[1][:mct])
                            nc.scalar.dma_start(
                                upd[ct * P:ct * P + mct, 512:1024],
                                usb[:mct, 512:1024])
                        else:
                            nc.scalar.activation(
                                usb[:mct, n * 512:(n + 1) * 512], pss[n][:mct],
                                AF.Copy,
                            )
                            nc.sync.dma_start(
                                upd[ct * P:ct * P + mct, n * 512:(n + 1) * 512],
                                usb[:mct, n * 512:(n + 1) * 512])
            ph_enc.release()

    nc.compile()
    return nc


def kernel(embedded, W1, b1, W2, b2, prototypes, label_tensor):
    global _CACHED_NC, LAST_RESULTS
    embedded = np.ascontiguousarray(np.asarray(embedded, dtype=np.float32))
    W1 = np.ascontiguousarray(np.asarray(W1, dtype=np.float32))
    b1 = np.asarray(b1, dtype=np.float32)
    W2 = np.ascontiguousarray(np.asarray(W2, dtype=np.float32))
    b2 = np.asarray(b2, dtype=np.float32)
    prototypes = np.ascontiguousarray(np.asarray(prototypes, dtype=np.float32))
    labels = np.asarray(label_tensor).astype(np.int64)

    # sort the batch by label; contiguous shards then have narrow label
    # spans, letting mm4 compute updates into a CWIN-wide class window
    perm = np.argsort(labels, kind="stable")
    labels_s = labels[perm]
    span_max = max(
        int(labels_s[c * BL + BL - 1] - labels_s[c * BL] + 1)
        for c in range(NCORES))
    variant = next((w for w in WINDOWS if span_max <= w), None)
    spans_ok = variant is not None
    if variant not in _CACHED:
        _CACHED[variant] = build_nc(cwin=variant)
    nc = _CACHED[variant]
    if spans_ok:
        embedded_use = embedded[perm]
        labels_use = labels_s
    else:
        embedded_use = embedded
        labels_use = labels

    pT = np.ascontiguousarray(prototypes.T)                      # [E, C]
    w1p = np.ascontiguousarray(
        W1.reshape(D // P, P, N2 // P, P).transpose(1, 2, 0, 3))  # [128,16,8,128]
    w2p = np.ascontiguousarray(
        W2.reshape(N2 // P, P, E // P, P).transpose(1, 2, 0, 3))  # [128,8,16,128]
    p2 = np.sum(prototypes.astype(np.float64) ** 2, axis=1).astype(np.float32)
    p2r = np.ascontiguousarray(np.broadcast_to(p2[None, :], (P, C)))
    b1c = np.ascontiguousarray(b1.reshape(N2 // P, P).T)          # [128, 16]
    b2c = np.ascontiguousarray(b2.reshape(E // P, P).T)           # [128, 8]

    in_maps = []
    c0s = []
    for c in range(NCORES):
        sl = slice(c * BL, (c + 1) * BL)
        xT = np.ascontiguousarray(embedded_use[sl].T)            # [D, BL]
        lab_c = labels_use[sl]
        c0 = int(lab_c.min()) if spans_ok else 0
        c0s.append(c0)
        labf = np.ascontiguousarray(
            (lab_c - c0).reshape(BL // P, P).T.astype(np.float32))  # [128, 8]
        in_maps.append({
            "xT": xT, "w1p": w1p, "w2p": w2p, "pT": pT, "p2r": p2r,
            "b1c": b1c, "b2c": b2c, "labf": labf,
        })

    res = run_bass_kernel_spmd(nc, in_maps, core_ids=list(range(NCORES)))
    LAST_RESULTS = res

    scores_cat = np.concatenate(
        [res.results[c]["scores"] for c in range(NCORES)], axis=0)
    if spans_ok:
        scores = np.empty_like(scores_cat)
        scores[perm] = scores_cat
        upd = np.zeros((C + CWIN, E), dtype=np.float64)
        for c in range(NCORES):
            upd[c0s[c]:c0s[c] + variant] += res.results[c]["upd"]
        upd = upd[:C]
    else:
        scores = scores_cat
        upd = np.zeros((C, E), dtype=np.float64)
        for c in range(NCORES):
            upd += res.results[c]["upd"]
    prototype_updates = upd.astype(np.float32)
    prototype_update_counts = np.bincount(labels, minlength=C).astype(np.float32)
    return scores, prototype_updates, prototype_update_counts


# revision 50
# speedup vs baseline: 1.0020x; 1.0020x over previous
"""DeepNCMDecoder Trainium2 kernel: 8-core data-parallel over batch.

Per core (batch shard BL=1024):
  h   = relu(x @ W1 + b1)           -- computed transposed: hT[n2, b]
  enc = h @ W2 + b2                 -- computed transposed: encT[e, b]
  scores = -max(x2 + p2 - 2 enc@P^T, 0) = min(2*s - x2 - p2, 0)
  updates = one_hot(labels).T @ enc   (partial; host sums across cores)
  counts  = bincount(labels)          (host; pure label metadata)

All matmuls run as fp32r (TF32-class, ~2e-4 rel err, full PE rate).
"""
import numpy as np
import concourse.bass as bass
import concourse.mybir as mybir
import concourse.tile as tile
from concourse import bacc
from concourse.bass_utils import run_bass_kernel_spmd
from concourse.masks import make_identity

F32 = mybir.dt.float32
F32R = mybir.dt.float32r
AF = mybir.ActivationFunctionType
ALU = mybir.AluOpType

B, D, E, C = 8192, 1024, 1024, 1000
N2 = 2 * E               # 2048
NCORES = 8
BL = B // NCORES         # 1024 rows per core
P = 128

_CACHED = {}             # variant -> compiled Bacc program
LAST_RESULTS = None      # test harness introspection
WINDOWS = (192, 256, 384)  # updates window ladder; host sorts by label
CWIN = 384               # widest windowed variant (for the host pad buffer)


def build_nc(phases="1234", cwin=192):
    nc = bacc.Bacc("TRN2", target_bir_lowering=False, debug=False,
                   num_devices=NCORES)

    xT = nc.dram_tensor("xT", [D, BL], F32, kind="ExternalInput")
    w1p = nc.dram_tensor("w1p", [P, N2 // P, D // P, P], F32, kind="ExternalInput")
    w2p = nc.dram_tensor("w2p", [P, E // P, N2 // P, P], F32, kind="ExternalInput")
    pT = nc.dram_tensor("pT", [E, C], F32, kind="ExternalInput")
    p2r = nc.dram_tensor("p2r", [P, C], F32, kind="ExternalInput")
    b1c = nc.dram_tensor("b1c", [P, N2 // P], F32, kind="ExternalInput")
    b2c = nc.dram_tensor("b2c", [P, E // P], F32, kind="ExternalInput")
    labf = nc.dram_tensor("labf", [P, BL // P], F32, kind="ExternalInput")

    CU = cwin if cwin else C          # updates window width
    scores = nc.dram_tensor("scores", [BL, C], F32, kind="ExternalOutput")
    upd = nc.dram_tensor("upd", [CU, E], F32, kind="ExternalOutput")

    KD = D // P          # 8  k-tiles for mm1
    MT = N2 // P         # 16 m-tiles (hT partition tiles)
    KN2 = N2 // P        # 16 k-tiles for mm2
    ET = E // P          # 8  e-tiles (encT partition tiles)
    BT = BL // P         # 8  b-tiles
    NB = BL // 512       # 2  n-tiles over b
    CH = C // 2          # 500 (c split for PSUM)
    CT_SIZES = [P] * (CU // P) + ([CU % P] if CU % P else [])

    with tile.TileContext(nc) as tc:
        with (
            tc.tile_pool(name="persist", bufs=1) as persist,
            tc.tile_pool(name="psum", bufs=2, space="PSUM") as psum,
        ):
            # --- persistent small tensors -------------------------------
            b1_sb = persist.tile([P, MT], F32, name="b1_sb")
            pT_sb = persist.tile([P, ET, C], F32R, name="pT_sb")
            b2_sb = persist.tile([P, ET], F32, name="b2_sb")
            encT = persist.tile([P, ET, E], F32R, name="encT")
            ident = persist.tile([P, P], F32, name="ident")
            x2pos = persist.tile([P, BT], F32, name="x2pos")

            do2 = "2" in phases
            do3 = "3" in phases
            do4 = "4" in phases

            ph_hT = tc.alloc_tile_pool(name="ph_hT", bufs=1)
            wpool = tc.alloc_tile_pool(name="wpool", bufs=4)
            hT = ph_hT.tile([P, MT, BL], F32R, name="hT")

            def load_w1(m):
                wt = wpool.tile([P, KN2, P], F32R, name="wt", tag="wt")
                nc.sync.dma_start(wt[:, :KD, :], w1p[:, m].bitcast(F32R))
                return wt

            def load_w2(e):
                wt = wpool.tile([P, KN2, P], F32R, name="wt", tag="wt")
                nc.sync.dma_start(wt, w2p[:, e].bitcast(F32R))
                return wt

            # --- mm1: hT = relu(W1.T x.T + b1) ----------------------
            ph_mm1 = tc.alloc_tile_pool(name="ph_mm1", bufs=1)
            # PE warmup: dummy matmuls during the startup DMA window keep the
            # PE continuously busy (model ramp + HW HAM) until real data lands
            wu = ph_mm1.tile([P, 512], F32, name="wu")
            nc.vector.memset(wu, 1.0)
            for i in range(8):
                wps = psum.tile([P, 512], F32, name=f"wps_{i}", tag="tps")
                nc.tensor.matmul(wps, wu[:, 0:P].bitcast(F32R),
                                 wu.bitcast(F32R), start=True, stop=True)
            xT_sb = ph_mm1.tile([P, KD, BL], F32R, name="xT_sb")
            xT_r = xT.rearrange("(o p) b -> p o b", p=P).bitcast(F32R)
            # startup-critical loads first, split for latency
            nc.sync.dma_start(xT_sb[:, 0, 0:512], xT_r[:, 0, 0:512])
            w0 = wpool.tile([P, KN2, P], F32R, name="wt", tag="wt")
            nc.sync.dma_start(w0[:, 0:4, :], w1p[:, 0, 0:4].bitcast(F32R))
            nc.sync.dma_start(w0[:, 4:KD, :], w1p[:, 0, 4:KD].bitcast(F32R))
            w_next = w0
            # n-major xT streaming: first halves of every k, then second halves
            for k in range(1, KD):
                nc.sync.dma_start(xT_sb[:, k, 0:512], xT_r[:, k, 0:512])
            wq = [w_next, load_w1(1), load_w1(2)]
            nc.sync.dma_start(b1_sb, b1c[:, :])
            nc.sync.dma_start(b2_sb, b2c[:, :])
            for k in range(KD):
                nc.sync.dma_start(xT_sb[:, k, 512:BL], xT_r[:, k, 512:BL])
            make_identity(nc, ident)
            for m in range(MT):
                w1t = wq.pop(0)
                if m + 3 < MT:
                    wq.append(load_w1(m + 3))
                pss = [psum.tile([P, 512], F32, name=f"psA_{m}_{n}",
                                 tag=f"ps{n}", bufs=3) for n in range(NB)]
                for n in range(NB):
                    for k in range(KD):
                        nc.tensor.matmul(
                            pss[n], w1t[:, k, :],
                            xT_sb[:, k, n * 512:(n + 1) * 512],
                            start=(k == 0), stop=(k == KD - 1),
                        )
                for n in range(NB):
                    nc.scalar.activation(
                        hT[:, m, n * 512:(n + 1) * 512], pss[n],
                        AF.Relu, bias=b1_sb[:, m:m + 1], scale=1.0,
                    )
            ph_mm1.release()

            # enc lives on the right side of the heap: outlives hT/wpool
            ph_enc = tc.alloc_tile_pool(name="ph_enc", bufs=1, side="right")
            enc = ph_enc.tile([P, BT, E], F32R, name="enc")

            def transpose_etile(e):
                # PE-transpose encT e-tile into enc columns (one per b-tile)
                for bt in range(BT):
                    tps = psum.tile([P, P], F32, name=f"tps_{e}_{bt}",
                                    tag="tps")
                    nc.tensor.transpose(
                        tps, encT[:, e, bt * P:(bt + 1) * P].bitcast(F32),
                        ident,
                    )
                    nc.vector.tensor_copy(
                        enc[:, bt, e * P:(e + 1) * P], tps)

            # --- mm2: encT = W2.T hT + b2, transposes interleaved ----
            if do2:
                pT_r = pT.rearrange("(o p) c -> p o c", p=P).bitcast(F32R)
                w2q = [load_w2(0), load_w2(1)]
                for e in range(ET):
                    w2t = w2q.pop(0)
                    if e + 2 < ET:
                        w2q.append(load_w2(e + 2))
                    nc.sync.dma_start(pT_sb[:, e], pT_r[:, e])
                    pss = [psum.tile([P, 512], F32, name=f"psB_{e}_{n}",
                                     tag=f"ps{n}", bufs=3) for n in range(NB)]
                    for k in range(KN2):
                        for n in range(NB):
                            nc.tensor.matmul(
                                pss[n], w2t[:, k, :],
                                hT[:, k, n * 512:(n + 1) * 512],
                                start=(k == 0), stop=(k == KN2 - 1),
                            )
                    for n in range(NB):
                        nc.scalar.activation(
                            encT[:, e, n * 512:(n + 1) * 512], pss[n],
                            AF.Identity, bias=b2_sb[:, e:e + 1], scale=1.0,
                        )
                    if do4 and e > 0:
                        transpose_etile(e - 1)
                if do4:
                    transpose_etile(ET - 1)
            wpool.release()
            ph_hT.release()

            # --- phase 2: x2, one-hot, scores, updates ------------------
            with tc.tile_pool(name="ph2", bufs=1) as ph2, \
                 tc.tile_pool(name="scratch", bufs=2) as scratch:
                oh = ph2.tile([P, BT, CU], F32R, name="oh")
                p2_sb = ph2.tile([P, C], F32, name="p2_sb")
                lab_sb = ph2.tile([P, BT], F32, name="lab_sb")
                iota_sb = ph2.tile([P, C], F32, name="iota_sb")

                nc.sync.dma_start(p2_sb, p2r[:, :])
                nc.sync.dma_start(lab_sb, labf[:, :])
                nc.gpsimd.iota(iota_sb, [[1, C]], channel_multiplier=0,
                               allow_small_or_imprecise_dtypes=True)

                def emit_x2(bt):
                    # x2 on ACT (Square + accum_out); emitted behind each
                    # bt's psum-draining copies so the ACT FIFO stays prompt
                    scr = scratch.tile([P, E], F32, name="scr", tag="scr")
                    nc.scalar.activation(
                        scr, enc[:, bt, :].bitcast(F32), AF.Square,
                        accum_out=x2pos[:, bt:bt + 1],
                    )

                if do4:
                    emit_x2(0)
                else:
                    nc.vector.memset(x2pos, 0.0)

                # mm3: scores
                def emit_oh(bt):
                    nc.vector.tensor_scalar(
                        oh[:, bt, :], iota_sb[:, :CU], lab_sb[:, bt:bt + 1],
                        None, ALU.is_equal,
                    )

                for bt in range(BT if do3 else 0):
                    if do4 and bt < 4:
                        emit_oh(2 * bt)
                        emit_oh(2 * bt + 1)
                    pss = [psum.tile([P, 512], F32, name=f"psC_{bt}_{ci}",
                                     tag=f"ps{ci}", bufs=3) for ci in range(2)]
                    for k in range(ET):
                        lhs = encT[:, k, bt * P:(bt + 1) * P]
                        for ci in range(2):
                            nc.tensor.matmul(
                                pss[ci][:, :CH], lhs,
                                pT_sb[:, k, ci * CH:(ci + 1) * CH],
                                start=(k == 0), stop=(k == ET - 1),
                            )
                    sco = scratch.tile([P, C], F32, name="sco", tag="sco",
                                       bufs=3)
                    for ci in range(2):
                        nc.scalar.activation(
                            sco[:, ci * CH:(ci + 1) * CH], pss[ci][:, :CH],
                            AF.Copy, scale=2.0,
                        )
                    if do4 and bt + 1 < BT:
                        emit_x2(bt + 1)
                    nc.vector.tensor_tensor(sco, sco, p2_sb, ALU.subtract)
                    nc.vector.tensor_scalar(sco, sco, x2pos[:, bt:bt + 1],
                                            0.0, ALU.subtract, ALU.min)
                    nc.sync.dma_start(scores[bt * P:(bt + 1) * P, :], sco)

                if do4 and not do3:
                    for bt in range(BT):
                        emit_oh(bt)

                # mm4: updates + counts
                for ct, mct in enumerate(CT_SIZES if do4 else []):
                    # ct=0 uses the tps banks (idle after transposes) so the
                    # first mm4 group doesn't wait on mm3's psum slots
                    pss = [psum.tile([P, 512], F32, name=f"psD_{ct}_{n}",
                                     tag=("tps" if ct == 0 else f"ps{n}"),
                                     bufs=(2 if ct == 0 else 3))
                           for n in range(2)]
                    for bt in range(BT):
                        lhs = oh[:, bt, ct * P:ct * P + mct]
                        for n in range(2):
                            nc.tensor.matmul(
                                pss[n][:mct], lhs,
                                enc[:, bt, n * 512:(n + 1) * 512],
                                start=(bt == 0), stop=(bt == BT - 1),
                            )
                    usb = scratch.tile([P, E], F32, name="usb", tag="usb")
                    last = ct == len(CT_SIZES) - 1
                    for n in range(2):
                        if last and n == 1:
                            # final tile: DVE (idle at kernel end) drains the
                            # second half in parallel with ACT's first half,
                            # and its store rides the ACT HWDGE ring so the
                            # two final DMA configs parallelize
                            nc.vector.tensor_copy(
                                usb[:mct, 512:1024], pss[1][:mct])
                            nc.scalar.dma_start(
                                upd[ct * P:ct * P + mct, 512:1024],
                                usb[:mct, 512:1024])
                        else:
                            nc.scalar.activation(
                                usb[:mct, n * 512:(n + 1) * 512], pss[n][:mct],
                                AF.Copy,
                            )
                            nc.sync.dma_start(
                                upd[ct * P:ct * P + mct, n * 512:(n + 1) * 512],
                                usb[:mct, n * 512:(n + 1) * 512])
            ph_enc.release()

    nc.compile()
    return nc


def kernel(embedded, W1, b1, W2, b2, prototypes, label_tensor):
    global _CACHED_NC, LAST_RESULTS
    embedded = np.ascontiguousarray(np.asarray(embedded, dtype=np.float32))
    W1 = np.ascontiguousarray(np.asarray(W1, dtype=np.float32))
    b1 = np.asarray(b1, dtype=np.float32)
    W2 = np.ascontiguousarray(np.asarray(W2, dtype=np.float32))
    b2 = np.asarray(b2, dtype=np.float32)
    prototypes = np.ascontiguousarray(np.asarray(prototypes, dtype=np.float32))
    labels = np.asarray(label_tensor).astype(np.int64)

    # sort the batch by label; contiguous shards then have narrow label
    # spans, letting mm4 compute updates into a CWIN-wide class window
    perm = np.argsort(labels, kind="stable")
    labels_s = labels[perm]
    span_max = max(
        int(labels_s[c * BL + BL - 1] - labels_s[c * BL] + 1)
        for c in range(NCORES))
    variant = next((w for w in WINDOWS if span_max <= w), None)
    spans_ok = variant is not None
    if variant not in _CACHED:
        _CACHED[variant] = build_nc(cwin=variant)
    nc = _CACHED[variant]
    if spans_ok:
        embedded_use = embedded[perm]
        labels_use = labels_s
    else:
        embedded_use = embedded
        labels_use = labels

    pT = np.ascontiguousarray(prototypes.T)                      # [E, C]
    w1p = np.ascontiguousarray(
        W1.reshape(D // P, P, N2 // P, P).transpose(1, 2, 0, 3))  # [128,16,8,128]
    w2p = np.ascontiguousarray(
        W2.reshape(N2 // P, P, E // P, P).transpose(1, 2, 0, 3))  # [128,8,16,128]
    p2 = np.sum(prototypes.astype(np.float64) ** 2, axis=1).astype(np.float32)
    p2r = np.ascontiguousarray(np.broadcast_to(p2[None, :], (P, C)))
    b1c = np.ascontiguousarray(b1.reshape(N2 // P, P).T)          # [128, 16]
    b2c = np.ascontiguousarray(b2.reshape(E // P, P).T)           # [128, 8]

    in_maps = []
    c0s = []
    for c in range(NCORES):
        sl = slice(c * BL, (c + 1) * BL)
        xT = np.ascontiguousarray(embedded_use[sl].T)            # [D, BL]
        lab_c = labels_use[sl]
        c0 = int(lab_c.min()) if spans_ok else 0
        c0s.append(c0)
        labf = np.ascontiguousarray(
            (lab_c - c0).reshape(BL // P, P).T.astype(np.float32))  # [128, 8]
        in_maps.append({
            "xT": xT, "w1p": w1p, "w2p": w2p, "pT": pT, "p2r": p2r,
            "b1c": b1c, "b2c": b2c, "labf": labf,
        })

    res = run_bass_kernel_spmd(nc, in_maps, core_ids=list(range(NCORES)))
    LAST_RESULTS = res

    scores_cat = np.concatenate(
        [res.results[c]["scores"] for c in range(NCORES)], axis=0)
    if spans_ok:
        scores = np.empty_like(scores_cat)
        scores[perm] = scores_cat
        upd = np.zeros((C + CWIN, E), dtype=np.float64)
        for c in range(NCORES):
            upd[c0s[c]:c0s[c] + variant] += res.results[c]["upd"]
        upd = upd[:C]
    else:
        scores = scores_cat
        upd = np.zeros((C, E), dtype=np.float64)
        for c in range(NCORES):
            upd += res.results[c]["upd"]
    prototype_updates = upd.astype(np.float32)
    prototype_update_counts = np.bincount(labels, minlength=C).astype(np.float32)
    return scores, prototype_updates, prototype_update_counts
